# revision 73
# baseline (speedup 1.0000x reference)
"""Dual-branch cross-attention block (nn_Attention) on 8 Trainium2 NeuronCores.

Sharding: pure data-parallel over batch B=8 - one batch element per core, no
collectives. Each core runs the full block for its element.

Design notes (cost-model driven):
  - fc and output projections are linear back-to-back, so they are composed
    host-side: Wco = Wfc @ Wo, bco = bfc @ Wo + bo. One 1536->768 projection
    instead of 1536->768 + 768->768.
  - everything bf16 on the matmul path (1 cycle/row at any N, half the HBM
    traffic, xbar DMA-transpose eligibility); psum stays f32.
  - inputs arrive transposed via dma_start_transpose (xbar tiles): no PE
    transposes, no DVE staging copies.
  - attention AV uses et (exp scores) as the *stationary* operand and the
    ones-augmented V as the *moving* operand (N=65 per matmul): 8x less PE
    streaming for that stage, and the output lands in [qpos, dh] layout where
    softmax normalization is per-partition: one reciprocal [128,4] + one
    broadcast multiply per (head, pattern).
  - the fused output projection consumes aT = [feature, qpos], produced by
    xbar DMA-transpose of the normalized attention output.
  - per-partition biases (q/k proj) fold into the DVE psum-drain add; free-dim
    biases (v, composed out) use a one-time PE ones-broadcast tile.
  - attention head-units are software-pipelined (scores/exp of head h emitted
    before AV of head h-1) and interleaved with the other branch's projection
    / fused-output matmul chunks so the scalar engine's exp stream overlaps
    PE work instead of serializing behind it.
"""

import numpy as np

import concourse.bass as bass
import concourse.mybir as mybir
from concourse.masks import make_identity
import concourse.tile as tile
from concourse import bacc
from concourse.bass_utils import run_bass_kernel_spmd

F32 = mybir.dt.float32
BF16 = mybir.dt.bfloat16
AF = mybir.ActivationFunctionType

B, S, D, H, DH = 8, 512, 768, 12, 64
KT = D // 128           # 6 k-tiles over D
FCKT = 2 * D // 128     # 12 k-tiles over 2D
ST = S // 128           # 4 s-tiles (128-row chunks of the 512 positions)

PROJ_W = ["Wq", "Wk", "Wv", "Wqm", "Wkm", "Wvm"]
TUNE = {"bias_pool": 1, "wu_a": 40, "act_epi": 1, "atm_act": 1, "av_pri": 2000, "wq_m0": 1, "wv_sp": 1, "mt_sp": 1}
COL_BIAS = ["bq", "bk", "bqm", "bkm"]      # bias along psum partitions
ROW_BIAS = ["bv", "bvm", "bco", "bcom"]    # bias along psum free dim


def build_program():
    nc = bacc.Bacc("TRN2", target_bir_lowering=False, debug=False, num_devices=8)

    x_h = nc.dram_tensor("x_h", [S, D], BF16, kind="ExternalInput")
    x_m = nc.dram_tensor("x_m", [S, D], BF16, kind="ExternalInput")
    wd = {n: nc.dram_tensor(n, [D, D], BF16, kind="ExternalInput") for n in PROJ_W}
    wd["Wco"] = nc.dram_tensor("Wco", [2 * D, D], BF16, kind="ExternalInput")
    wd["Wcom"] = nc.dram_tensor("Wcom", [2 * D, D], BF16, kind="ExternalInput")
    bcols_d = nc.dram_tensor("bcols", [1, 4 * D], F32, kind="ExternalInput")
    brows_d = nc.dram_tensor("brows", [1, 4 * D], BF16, kind="ExternalInput")
    out_p = nc.dram_tensor("out_p", [S, D], F32, kind="ExternalOutput")
    out_m = nc.dram_tensor("out_m", [S, D], F32, kind="ExternalOutput")

    with tile.TileContext(nc) as tc:
        with tc.tile_pool(name="cst", bufs=1) as cst, \
             tc.tile_pool(name="persist", bufs=1) as pp, \
             tc.tile_pool(name="w768", bufs=4) as wp, \
             tc.tile_pool(name="bias", bufs=1) as biasp, \
             tc.tile_pool(name="et", bufs=TUNE.get("etb", 24)) as etp, \
             tc.tile_pool(name="abuf", bufs=2) as ap_pool, \
             tc.tile_pool(name="aT", bufs=2) as atp, \
             tc.tile_pool(name="ot", bufs=TUNE.get("otb", 4)) as otp, \
             tc.tile_pool(name="rc", bufs=TUNE.get("rcb", 4)) as rcp, \
             tc.tile_pool(name="ps", bufs=2, space="PSUM") as ps, \
             tc.tile_pool(name="pj", bufs=2, space="PSUM") as pj, \
             tc.tile_pool(name="av", bufs=2, space="PSUM") as avps:

            # ---------------- phase A: input staging ----------------
            # packed bias loads first (2 tiny DMAs), then xt/wk pieces
            # interleaved so the first projection chunk starts ~4us in, then
            # the remaining weights in consumption order

            def load_w(name, dram_slice=None, pool=False, act=False, sp=False):
                t = wp.tile([128, KT, D], BF16, tag="w768")
                src = (dram_slice if dram_slice is not None else wd[name][:, :]) \
                    .rearrange("(ko ki) m -> ki ko m", ki=128)
                eng = nc.gpsimd if pool else (nc.scalar if act else nc.sync)
                if sp:
                    eng.dma_start(t[:, :, 0:384], src[:, :, 0:384])
                    eng.dma_start(t[:, :, 384:768], src[:, :, 384:768])
                else:
                    eng.dma_start(t[:], src)
                return t

            # merged transfers in strict consumption order: the per-queue DMA
            # pipeline only keeps 2 in flight (issue N waits completion of
            # N-2), so fewer/larger transfers reach the consumers soonest
            xt = pp.tile([128, KT, S], BF16, tag="xt")
            nc.sync.dma_start_transpose(xt[:], x_h[:, :])
            wk = load_w("Wk", pool=TUNE.get("wk_pool", 0), act=TUNE.get("wk_act", 0))
            beng = nc.gpsimd if TUNE.get("bias_pool", 1) else nc.sync
            bcall = biasp.tile([128, 4, KT], F32, tag="bcall")
            beng.dma_start(bcall[:], bcols_d.rearrange("one (n m p) -> (one p) n m",
                                                       p=128, n=4))
            brall = biasp.tile([1, 4 * D], BF16, tag="brall")
            beng.dma_start(brall[:], brows_d[:])
            bq_c, bk_c, bqm_c, bkm_c = (bcall[:, 0, :], bcall[:, 1, :],
                                        bcall[:, 2, :], bcall[:, 3, :])
            brow_tiles = {n: brall[:, i * D:(i + 1) * D]
                          for i, n in enumerate(ROW_BIAS)}
            mt = pp.tile([128, KT, S], BF16, tag="mt")
            if TUNE.get("mt_sp", 0):
                nc.sync.dma_start_transpose(mt[:, 0:3, :], x_m[:, 0:384])
                nc.sync.dma_start_transpose(mt[:, 3:6, :], x_m[:, 384:768])
            else:
                nc.sync.dma_start_transpose(mt[:], x_m[:, :])
            if TUNE.get("wqm_m0", 0):
                wqm = wp.tile([128, KT, D], BF16, tag="w768", name="wqm")
                wqm_src = wd["Wqm"][:, :].rearrange("(ko ki) m -> ki ko m", ki=128)
                nc.sync.dma_start(wqm[:, :, 0:128], wqm_src[:, :, 0:128])
                wqm_rest = lambda: nc.sync.dma_start(wqm[:, :, 128:768],
                                                     wqm_src[:, :, 128:768])
            else:
                wqm = load_w("Wqm", act=TUNE.get("wqm_act", 0))
                wqm_rest = None
            if TUNE.get("wq_m0", 0):
                wq = wp.tile([128, KT, D], BF16, tag="w768", name="wq")
                wq_src = wd["Wq"][:, :].rearrange("(ko ki) m -> ki ko m", ki=128)
                nc.sync.dma_start(wq[:, :, 0:128], wq_src[:, :, 0:128])
                nc.sync.dma_start(wq[:, :, 128:768], wq_src[:, :, 128:768])
            else:
                wq = load_w("Wq")
            if wqm_rest is not None:
                wqm_rest()
            if TUNE.get("wv_sp", 0):
                wv = wp.tile([128, KT, D], BF16, tag="w768", name="wv")
                wv_src = wd["Wv"][:, :].rearrange("(ko ki) m -> ki ko m", ki=128)
                nc.sync.dma_start(wv[:, :, 0:384], wv_src[:, :, 0:384])
                nc.sync.dma_start(wv[:, :, 384:768], wv_src[:, :, 384:768])
            else:
                wv = load_w("Wv")
            wkm = load_w("Wkm", sp=TUNE.get("wk2_sp", 0))
            wvm = load_w("Wvm", sp=TUNE.get("wk2_sp", 0))

            ones_f = cst.tile([1, 128], F32)
            nc.vector.memset(ones_f[:], 1.0)
            ones = cst.tile([1, 128], BF16)
            nc.vector.tensor_copy(out=ones[:], in_=ones_f[:])

            # p-state warmup: dependency-free matmuls keep the PE streaming
            # while the first DMAs land, so real work starts at full clock
            warm_src = cst.tile([1, 512], BF16)
            nc.vector.memset(warm_src[:], 0.0)
            def warmup(n):
                for _ in range(n):
                    wpt = ps.tile([128, 2, 512], F32, tag="big")
                    nc.tensor.matmul(wpt[:, 0, :], warm_src[:, 0:128], warm_src[:],
                                     start=True, stop=True)

            def bias_bcast(n):
                """[1,768] bias -> [128,768] bf16 sbuf tile via PE ones-broadcast."""
                br = brow_tiles[n]
                pt = ps.tile([128, 2, 512], F32, tag="big")
                nc.tensor.matmul(pt[:, 0, :], ones[:], br[:, 0:512], start=True, stop=True)
                nc.tensor.matmul(pt[:, 1, 0:256], ones[:], br[:, 512:768], start=True, stop=True)
                bb = biasp.tile([128, D], BF16, tag="bb_" + n)
                flat = pt.rearrange("p a b -> p (a b)")
                nc.vector.tensor_copy(out=bb[:], in_=flat[:, 0:768])
                return bb

            warmup(TUNE.get("wu_a", 16))
            bvb = bias_bcast("bv")
            bvmb = bias_bcast("bvm")
            bcob = bias_bcast("bco")
            bcomb = bias_bcast("bcom")

            # ------------- transposed projection: yT[f, q] = W.T @ xT + b -------------
            def proj_T_chunk(w, bcol, src_t, dst, m, act_epi=False):
                pt = pj.tile([128, 512], F32, tag="pj")
                for k in range(KT):
                    nc.tensor.matmul(pt[:], w[:, k, m * 128:(m + 1) * 128],
                                     src_t[:, k, :], start=(k == 0), stop=(k == KT - 1))
                if act_epi and TUNE.get("act_epi", 1):
                    nc.scalar.activation(dst, pt[:], AF.Identity,
                                         bias=bcol[:, m:m + 1], scale=1.0)
                else:
                    nc.vector.tensor_scalar_add(out=dst, in0=pt[:],
                                                scalar1=bcol[:, m:m + 1])

            # ------------- v-projection, natural layout + ones column -------------
            def proj_vaug_chunk(w, bvb, src_t, va, st):
                for c in range(2):
                    pt = pj.tile([128, 512], F32, tag="pj")
                    for k in range(KT):
                        nc.tensor.matmul(pt[:, 0:384],
                                         src_t[:, k, st * 128:(st + 1) * 128],
                                         w[:, k, c * 384:(c + 1) * 384],
                                         start=(k == 0), stop=(k == KT - 1))
                    nc.vector.tensor_tensor(
                        out=va[:, st, c * 6:(c + 1) * 6, 0:DH],
                        in0=pt[:, 0:384].rearrange("p (h d) -> p h d", d=DH),
                        in1=bvb[:, c * 384:(c + 1) * 384].rearrange("p (h d) -> p h d", d=DH),
                        op=mybir.AluOpType.add)

            qq = pp.tile([128, KT, 2, S], BF16, tag="qq")
            kt = pp.tile([128, KT, S], BF16, tag="kt")
            kmt = pp.tile([128, KT, S], BF16, tag="kmt")
            vaug = pp.tile([128, ST, H, DH + 1], BF16, tag="vaug")
            nc.vector.memset(vaug[:, :, :, DH], 1.0)
            vmaug = pp.tile([128, ST, H, DH + 1], BF16, tag="vmaug")
            nc.vector.memset(vmaug[:, :, :, DH], 1.0)

            # ------------- attention unit, split for software pipelining ---------
            def attn_scores(h, k_src):
                b0 = (h % 2) * 64
                ko = h // 2
                ets = []
                with tc.high_priority(offset=TUNE.get("sc_pri", 4000)):
                    for i in range(ST):
                        sp = ps.tile([128, 2, 512], F32, tag="big")
                        for pat in range(2):
                            nc.tensor.matmul(sp[:, pat, :],
                                             k_src[b0:b0 + 64, ko, i * 128:(i + 1) * 128],
                                             qq[b0:b0 + 64, ko, pat, :],
                                             start=True, stop=True)
                        et = etp.tile([128, 2, S], BF16, tag="et")
                        nc.scalar.activation(et[:], sp[:], AF.Exp, scale=1.0 / 8.0)
                        ets.append(et)
                return ets

            # pat_cols: feature-column base for q (pat=0) / qm (pat=1) patterns:
            # branch p stores [a_pp | a_mp] -> (0, D); branch m [a_mm | a_pm] -> (D, 0)
            def attn_av(h, ets, v_src, a_dst, pat_cols):
              import contextlib
              with tc.high_priority(offset=TUNE.get("av_pri", 2000)) if TUNE.get("av_pri", 2000) \
                      else contextlib.nullcontext():
                for pat in range(2):
                    avp = avps.tile([128, ST, DH + 1], F32, tag="av")
                    for qc in range(ST):
                        for i in range(ST):
                            nc.tensor.matmul(avp[:, qc, :],
                                             ets[i][:, pat, qc * 128:(qc + 1) * 128],
                                             v_src[:, i, h, :],
                                             start=(i == 0), stop=(i == ST - 1))
                    rc = rcp.tile([128, ST], F32, tag="rc")
                    with nc.allow_low_precision(reason="softmax reciprocal"):
                        nc.vector.reciprocal(rc[:], avp[:, :, DH])
                    col = pat_cols[pat] + h * DH
                    nc.vector.tensor_tensor(
                        out=a_dst[:, :, col:col + DH],
                        in0=avp[:, :, 0:DH],
                        in1=rc[:, :, None].broadcast_to([128, ST, DH]),
                        op=mybir.AluOpType.mult)

            # ------------- fused (fc @ out) projection, two filler halves -------
            def out_half(wcoA, wcoB, bcob, aT, qc, c, off, cw, ot, out_dram,
                         act_dma=False):
                pt = pj.tile([128, 512], F32, tag="pj")
                for kf in range(FCKT):
                    w = wcoA if kf < KT else wcoB
                    nc.tensor.matmul(pt[:, 0:cw],
                                     aT[:, kf, qc * 128:(qc + 1) * 128],
                                     w[:, kf % KT, off:off + cw],
                                     start=(kf == 0), stop=(kf == FCKT - 1))
                nc.vector.tensor_tensor(out=ot[:, off:off + cw], in0=pt[:, 0:cw],
                                        in1=bcob[:, off:off + cw], op=mybir.AluOpType.add)
                eng = nc.scalar if act_dma else nc.sync
                eng.dma_start(out_dram[qc * 128:(qc + 1) * 128, off:off + cw],
                              ot[:, off:off + cw])

            def out_chunks(wcoA, wcoB, bcob, aT, out_dram, act_dma=False):
                chunks = []
                for qc in range(ST):
                    ot = otp.tile([128, D], F32, tag="ot")
                    for c, off, cw in ((0, 0, 512), (1, 512, 256)):
                        chunks.append(lambda qc=qc, c=c, off=off, cw=cw, ot=ot:
                                      out_half(wcoA, wcoB, bcob, aT, qc, c, off, cw,
                                               ot, out_dram,
                                               act_dma=act_dma and (qc % 2 == 1)))
                return chunks

            def attn_branch(k_src, v_src, a_dst, pat_cols, chunks, per_head=2,
                            depth=1, pre_at=None, chunk_from=1):
                """Depth-pipelined attention: scores/exp of head h, filler
                chunks, then AV of head h-depth (whose exps are in flight).
                pre_at[h]: extra chunks emitted right after scores of head h."""
                ci = 0
                for h in range(H):
                    attn_ets[h] = attn_scores(h, k_src)
                    for c in (pre_at or {}).get(h, ()):
                        c()
                    if h >= chunk_from:
                        for _ in range(per_head):
                            if ci < len(chunks):
                                chunks[ci]()
                                ci += 1
                    if h >= depth:
                        attn_av(h - depth, attn_ets.pop(h - depth), v_src, a_dst, pat_cols)
                while ci < len(chunks):
                    chunks[ci]()
                    ci += 1
                for h in range(H - depth, H):
                    attn_av(h, attn_ets.pop(h), v_src, a_dst, pat_cols)

            attn_ets = {}

            # ============ flat hand-tuned schedule =============================
            # Emission order IS the per-engine execution order; this schedule
            # interleaves projection / fused-output chunks between attention
            # head-units so the PE never waits on the exp stream or the DMA
            # queue, and scores stay 3-4 heads ahead of the AV consumers.
            ktc = lambda m, ae=False: proj_T_chunk(wk, bk_c, xt, kt[:, m, :], m, ae)
            qmc = lambda m, ae=False: proj_T_chunk(wqm, bqm_c, mt, qq[:, m, 1, :], m, ae)
            qqc = lambda m, ae=False: proj_T_chunk(wq, bq_c, xt, qq[:, m, 0, :], m, ae)
            kmc = lambda m: proj_T_chunk(wkm, bkm_c, mt, kmt[:, m, :], m)
            vac = lambda st: proj_vaug_chunk(wv, bvb, xt, vaug, st)
            vmc = lambda st: proj_vaug_chunk(wvm, bvmb, mt, vmaug, st)

            a_p = ap_pool.tile([128, ST, 2 * D], BF16, tag="a")
            a_m = ap_pool.tile([128, ST, 2 * D], BF16, tag="a")

            def S_p(h):
                attn_ets[("p", h)] = attn_scores(h, kt)

            def AV_p(h):
                attn_av(h, attn_ets.pop(("p", h)), vaug, a_p, (0, D))

            def S_m(h):
                attn_ets[("m", h)] = attn_scores(h, kmt)

            def AV_m(h):
                attn_av(h, attn_ets.pop(("m", h)), vmaug, a_m, (D, 0))

            # ---- phase B: prot attention + all remaining projections
            for m in range(KT):
                ktc(m, True)
            qmc(0, True); qmc(1, True); qmc(2, True)
            qqc(0, True)
            S_p(0); S_p(1)
            qmc(3); qmc(4)
            qqc(1)
            S_p(2)
            qmc(5); qqc(2)
            S_p(3); S_p(4)
            if TUNE.get("s5_early", 0):
                S_p(5)
            vac(0); vac(1); vac(2); vac(3)
            AV_p(0)
            if not TUNE.get("s5_early", 0):
                S_p(5)
            qqc(3)
            AV_p(1)
            S_p(6)
            qqc(4)
            AV_p(2)
            S_p(7)
            qqc(5)
            AV_p(3)
            S_p(8)
            kmc(0); kmc(1)
            AV_p(4)
            S_p(9)
            kmc(2); kmc(3)
            AV_p(5)
            S_p(10)
            kmc(4); kmc(5)
            AV_p(6)
            S_p(11)
            vmc(0)
            AV_p(7)
            vmc(1)
            AV_p(8)
            vmc(2)
            AV_p(9)
            vmc(3)
            S_m(0)
            AV_p(10)
            S_m(1)
            AV_p(11)

            # ---- b/c boundary: a_p transposes + fused-out weights
            aTp = atp.tile([128, FCKT, S], BF16, tag="aT")
            for qc in range(ST):
                teng = nc.scalar if (TUNE.get("at_act", 0) and qc % 2) else nc.sync
                teng.dma_start_transpose(aTp[:, :, qc * 128:(qc + 1) * 128],
                                         a_p[:, qc, :])
            wcoA = load_w("Wco", wd["Wco"][0:D, :], sp=TUNE.get("wco_sp", 0))
            wcoB = load_w("Wco", wd["Wco"][D:2 * D, :], sp=TUNE.get("wco_sp", 0))
            p_halves = out_chunks(wcoA, wcoB, bcob, aTp, out_p)

            wcom = {}

            def load_wcom():
                wcom["A"] = load_w("Wcom", wd["Wcom"][0:D, :], sp=TUNE.get("wco_sp", 0),
                                   pool=TUNE.get("wcom_pool", 0))
                wcom["B"] = load_w("Wcom", wd["Wcom"][D:2 * D, :], sp=TUNE.get("wco_sp", 0),
                                   pool=TUNE.get("wcom_pool", 0))


            # ---- phase C: mol attention + branch-p fused out
            S_m(2)
            p_halves[0]()
            S_m(3)
            p_halves[1]()
            AV_m(0)
            S_m(4)
            p_halves[2]()
            AV_m(1)
            S_m(5)
            p_halves[3]()
            AV_m(2)
            S_m(6)
            p_halves[4]()
            AV_m(3)
            S_m(7)
            p_halves[5]()
            AV_m(4)
            S_m(8)
            load_wcom()
            AV_m(5)
            S_m(9)
            AV_m(6)
            S_m(10)
            AV_m(7)
            S_m(11)
            AV_m(8); AV_m(9); AV_m(10); AV_m(11)

            # ---- phase D: a_m transposes + branch-m fused out
            aTm = atp.tile([128, FCKT, S], BF16, tag="aT")
            for qc in range(ST):
                teng = nc.scalar if (TUNE.get("atm_act", 0) and qc % 2) else nc.sync
                teng.dma_start_transpose(aTm[:, :, qc * 128:(qc + 1) * 128],
                                         a_m[:, qc, :])
            p_halves[6]()
            p_halves[7]()
            warmup(TUNE.get("wu_d", 0))
            for ch in out_chunks(wcom["A"], wcom["B"], bcomb, aTm, out_m,
                                 act_dma=bool(TUNE.get("om_act", 0))):
                ch()


    nc.compile()
    return nc


_PROGRAM_CACHE = {}


def prepare_in_maps(inputs):
    """Full-input dict -> per-core in_maps with host-side dtype prep and
    fc/out weight composition."""
    import ml_dtypes
    bf16 = lambda a: np.ascontiguousarray(np.asarray(a, np.float32)).astype(ml_dtypes.bfloat16)
    f64 = lambda n: np.asarray(inputs[n], np.float64)
    shared = {}
    for n in PROJ_W:
        shared[n] = bf16(inputs[n])
    shared["Wco"] = bf16(f64("Wfc") @ f64("Wo"))
    shared["Wcom"] = bf16(f64("Wfcm") @ f64("Wom"))
    shared["bco"] = bf16((f64("bfc") @ f64("Wo") + f64("bo")).reshape(1, D))
    shared["bcom"] = bf16((f64("bfcm") @ f64("Wom") + f64("bom")).reshape(1, D))
    shared["bcols"] = np.ascontiguousarray(np.concatenate(
        [np.asarray(inputs[n], np.float32).reshape(-1) for n in COL_BIAS])).reshape(1, 4 * D)
    shared["brows"] = np.concatenate(
        [np.asarray(shared.pop(n)).reshape(-1) if n in shared else bf16(inputs[n].reshape(1, D)).reshape(-1)
         for n in ROW_BIAS]).reshape(1, 4 * D)
    hs = bf16(inputs["hidden_states"].reshape(B, S, D))
    ml = bf16(inputs["mol"].reshape(B, S, D))
    return [dict(shared, x_h=np.ascontiguousarray(hs[b]),
                 x_m=np.ascontiguousarray(ml[b])) for b in range(B)]


def kernel(hidden_states, mol, Wq, bq, Wk, bk, Wv, bv, Wqm, bqm, Wkm, bkm,
           Wvm, bvm, Wfc, bfc, Wfcm, bfcm, Wo, bo, Wom, bom):
    if "nc" not in _PROGRAM_CACHE:
        _PROGRAM_CACHE["nc"] = build_program()
    nc = _PROGRAM_CACHE["nc"]
    in_maps = prepare_in_maps(dict(
        hidden_states=hidden_states, mol=mol, Wq=Wq, bq=bq, Wk=Wk, bk=bk,
        Wv=Wv, bv=bv, Wqm=Wqm, bqm=bqm, Wkm=Wkm, bkm=bkm, Wvm=Wvm, bvm=bvm,
        Wfc=Wfc, bfc=bfc, Wfcm=Wfcm, bfcm=bfcm, Wo=Wo, bo=bo, Wom=Wom, bom=bom))

    res = run_bass_kernel_spmd(nc, in_maps, core_ids=list(range(B)))
    attn_prot = np.stack([res.results[b]["out_p"] for b in range(B)])
    attn_mol = np.stack([res.results[b]["out_m"] for b in range(B)])
    return attn_prot, attn_mol
